# revision 25
# baseline (speedup 1.0000x reference)
"""3D Haar DWT (2x2x2 blocks, 8 subbands) on 8 Trainium2 NeuronCores.

Input  x: (2, 16, 64, 128, 128) f32.
Output: tuple of 8 subbands, each (2, 16, 32, 64, 64) f32, subband order
LLL,LLH,LHL,LHH,HLL,HLH,HHL,HHH (filters applied to (D,H,W) resp.).

Strategy (pure data parallel, zero cross-core communication), v2 = bf16:
  - The 2e-2 rel-err budget admits bf16 I/O (measured ~2e-3), halving HBM
    traffic per core to 8 MiB in + 8 MiB out -> ~47us DMA roofline/core.
  - Flatten (B,C) -> 32 slabs of (64,128,128); core i takes 4 = 2 PAIRS.
    A pair of slabs is processed together with SBUF partitions = (d, tp)
    (tp = slab parity).  Compared to (d, h-half) partitions this keeps
    per-partition DRAM runs at 16KB on input and 8KB on output (the h
    dimension stays whole in the free dim), so every DMA descriptor is
    large.
  - TensorEngine does the D-axis AND H-axis butterflies: the constant
    128x128 matrix M (and its negation) does the D butterfly on the
    partition axis, and PSUM accumulation over the two H-parity column
    sets does the H butterfly:
      u0 = M @ x[q=0] + M @ x[q=0],  u1 = M @ x[q=0] + (-M) @ x[q=1].
    The moving-operand access patterns feed columns in (hb, r, w2) order,
    so PSUM lands with the W-axis parity r DEINTERLEAVED.
  - ScalarE drains PSUM f32 -> SBUF bf16 (cast on copy) -- the only engine
    that can, at 1x (PSUM has one read port); it is the pipeline pacer.
  - DVE only does the W butterfly (pair r blocks, stride-1, bf16 2x mode).
  - A garbage-operand matmul warmup burst holds the PE busy ~4us up front
    so the HAM clock gate is at 2.4 GHz when real matmuls arrive.
  - Input DMAs on SP (HWDGE), output DMAs on GPSIMD (SWDGE): each ring has
    one producer so issues never queue behind each other.
"""

import numpy as np
import ml_dtypes

_B, _C, _D, _H, _W = 2, 16, 64, 128, 128
_NCORES = 8
_SLABS = _B * _C  # 32
_SLABS_PER_CORE = _SLABS // _NCORES  # 4
_PAIRS = _SLABS_PER_CORE // 2  # 2

_BF16 = ml_dtypes.bfloat16


def _haar_filters_np():
    # Bit-identical construction to the reference filter bank.
    s = 1.0 / np.sqrt(2.0)
    L = np.array([s, s], dtype=np.float32)
    H = np.array([s, -s], dtype=np.float32)
    bands = [(a, b, c) for a in "LH" for b in "LH" for c in "LH"]
    filt = np.stack(
        [
            (L if a == "L" else H)[:, None, None]
            * (L if b == "L" else H)[None, :, None]
            * (L if c == "L" else H)[None, None, :]
            for (a, b, c) in bands
        ],
        axis=0,
    )  # (8, 2, 2, 2) float32
    return filt


def _haar_matrix():
    """(128,128) for the D-axis butterfly on the partition axis.

    Input partition  = d*2 + tp          (tp = slab parity, d = depth 0..63)
    Output partition = tp*64 + dp*2 + a  (a = D band, dp = 0..31)
    (tp-major over dp so the output DMA's DRAM-side (tp, dp) dims merge into
    one 64-long OUTER dim -- 3-dim AP and a full 16-engine SDMA spray; an
    outer dim of 2 would put the whole transfer on 2 engines.)
    Entry = f_a[p] * s * s  (d = 2dp+p): the full 1/(2*sqrt2) magnitude is
    folded here so the H/W butterflies on DVE are pure +/- adds."""
    filt = _haar_filters_np()
    M = np.zeros((128, 128), dtype=np.float32)
    for tp in range(2):
        for a in range(2):
            for dp in range(32):
                for p in range(2):
                    M[(2 * dp + p) * 2 + tp, tp * 64 + dp * 2 + a] = filt[
                        a * 4, p, 0, 0
                    ]
    return M


def _build_bass():
    import concourse.mybir as mybir
    import concourse.tile as tile
    from concourse import bacc

    f32 = mybir.dt.float32
    bf16 = mybir.dt.bfloat16
    nc = bacc.Bacc("TRN2", target_bir_lowering=False, debug=False)

    x = nc.dram_tensor("x", [_SLABS_PER_CORE, _D, _H, _W], bf16, kind="ExternalInput")
    hm = nc.dram_tensor("hm", [128, 256], bf16, kind="ExternalInput")  # [M | -M]
    y = nc.dram_tensor(
        "y", [8, _SLABS_PER_CORE, _D // 2, _H // 2, _W // 2], bf16,
        kind="ExternalOutput",
    )

    # x[t=2pr+tp, d, h, w] with h = hh*64 + hb*2 + q, w = w2*2 + r.
    # Half-pair tile (pr, hh): partitions (d, tp), free (hb, q, w) -- each
    # partition's free dim walks a CONTIGUOUS 16KB HBM region; split into two
    # 1MB DMAs (hb halves, 8KB/partition each) so matmuls start early.
    xr = x[:, :, :, :].rearrange(
        "(pr tp) d (hh hb q) w -> pr d tp hh hb q w", tp=2, hh=2, hb=32, q=2
    )
    # y[s=(a,b,g), t=2pr+tp, dp, hp=(hh,hb), wp]; partition order (tp, dp, a);
    # one DMA per (b, g, pr, hh): DRAM dims ((tp dp):64, a:2, (hb wp):2048) --
    # 4KB contiguous per partition, 64-long outer dim for the engine spray.
    yv = y[:, :, :, :, :].rearrange(
        "(a b g) (pr tp) dp (hh hb) wp -> b g pr hh tp dp a hb wp",
        a=2, b=2, tp=2, hh=2,
    )

    with tile.TileContext(nc) as tc:
        with (
            tc.tile_pool(name="const", bufs=1) as cpool,
            tc.tile_pool(name="xin", bufs=3) as xpool,
            tc.tile_pool(name="uband", bufs=2) as upool,
            tc.tile_pool(name="outs", bufs=3) as opool,
            tc.tile_pool(name="psum", bufs=2, space="PSUM") as ppool,
        ):
            hmt = cpool.tile([128, 256], bf16, tag="hm")
            nc.sync.dma_start(out=hmt[:, :], in_=hm[:, :])
            hmp, hmn = hmt[:, 0:128], hmt[:, 128:256]

            # PE warmup: ~3us of garbage matmuls flips the HAM clock gate
            # to 8/8 (2.4 GHz) before the first real matmul; operands are a
            # memset tile so the burst starts at t~0, gated by nothing.
            junk = cpool.tile([128, 640], bf16, tag="junk")
            nc.vector.memset(junk[:, :], 0.0)
            wp = ppool.tile([128, 2048], f32, tag="pq", name="warm")
            for i in range(14):
                nc.tensor.matmul(
                    wp[:, 0:256], junk[:, 0:128], junk[:, 128:384],
                    start=True, stop=True,
                )

            def load_half_pair(pr, hh, nsplit=2):
                xh = xpool.tile([128, 8192], bf16, tag="xt", name=f"xt_{pr}_{hh}")
                w = 32 // nsplit
                for c in range(nsplit):
                    nc.sync.dma_start(
                        out=xh[:, c * w * 256 : (c + 1) * w * 256],
                        in_=xr[pr, :, :, hh, c * w : (c + 1) * w],
                    )
                return xh

            HP = [(pr, hh) for pr in range(_PAIRS) for hh in range(2)]
            # First tile in quarters so the first matmul chain starts ~1MB
            # earlier; the rest in halves (bigger descriptors).
            tiles = {HP[0]: load_half_pair(*HP[0], nsplit=4),
                     HP[1]: load_half_pair(*HP[1])}

            for k, (pr, hh) in enumerate(HP):
                if k + 2 < len(HP):
                    tiles[HP[k + 2]] = load_half_pair(*HP[k + 2])
                xt = tiles.pop((pr, hh))

                # Engine balance: half-pair 0 computes the H butterfly on DVE
                # (which would otherwise idle at the start); the rest fold H
                # into the PE via PSUM accumulation (2x matmul passes).
                h_on_dve = k == 0

                # Moving-operand APs, H parity q split out, remaining column
                # order (hb, r, w2): PSUM lands W-deinterleaved.
                xv = xt[:, :].rearrange(
                    "m (hb q w2 r) -> m q hb r w2", hb=32, q=2, r=2
                )
                # Single-pass variant: columns in (hb, q, r, w2) order.
                xv2 = xt[:, :].rearrange(
                    "m (hb q w2 r) -> m hb q r w2", hb=32, q=2, r=2
                )
                # u tile: post D+H data, free (mq: mini-quad 4, b: H band 2,
                # hb: 8, r: 2, w2: 64) -- one ScalarE drain per mini-quad.
                ub = upool.tile([128, 8192], bf16, tag="ub", name=f"ub_{pr}_{hh}")
                o = [
                    [
                        opool.tile(
                            [128, 2048], bf16, tag=f"o{b}{g}",
                            name=f"o{b}{g}_{pr}_{hh}",
                        )
                        for g in range(2)
                    ]
                    for b in range(2)
                ]

                if h_on_dve:
                    # ub holds P (pre-H) in (mq, hb, q, r, w2) layout; DVE's
                    # H butterfly (pair q blocks) writes ud in the standard
                    # (mq, b, hb, r, w2) layout the W stage expects.
                    ud = upool.tile(
                        [128, 8192], bf16, tag="ud", bufs=1, name=f"ud_{pr}_{hh}"
                    )
                    uv = ud[:, :].rearrange(
                        "m (mq b hb r w2) -> m mq b hb r w2", mq=4, b=2, hb=8, r=2
                    )
                    pv = ub[:, :].rearrange(
                        "m (mq hb q rw) -> m mq hb q rw", mq=4, hb=8, q=2
                    )
                    udh = ud[:, :].rearrange(
                        "m (mq b hb rw) -> m mq b hb rw", mq=4, b=2, hb=8
                    )
                else:
                    uv = ub[:, :].rearrange(
                        "m (mq b hb r w2) -> m mq b hb r w2", mq=4, b=2, hb=8, r=2
                    )

                def h_stage(s):
                    # H butterfly on DVE for semi s (h_on_dve half-pairs):
                    # pair q=0/q=1 blocks of (r, w2)=128, stride-1 bf16 (2x).
                    ev = pv[:, 2 * s : 2 * s + 2, :, 0, :]
                    od = pv[:, 2 * s : 2 * s + 2, :, 1, :]
                    u0 = udh[:, 2 * s : 2 * s + 2, 0, :, :]
                    u1 = udh[:, 2 * s : 2 * s + 2, 1, :, :]
                    nc.vector.tensor_add(u0, ev, od)
                    nc.vector.tensor_sub(u1, ev, od)

                last_hp = k == len(HP) - 1

                def w_stage(s):
                    # W butterfly on DVE for semi s (mini-quads 2s, 2s+1):
                    # pair r=0/r=1 blocks of w2=64, all stride-1 bf16 (2x).
                    # On the last half-pair run per-mini-quad so the final op
                    # chain after the last PSUM drain is as short as possible.
                    if h_on_dve:
                        h_stage(s)
                    mqs = (
                        [(2 * s, 1), (2 * s + 1, 1)] if last_hp else [(2 * s, 2)]
                    )
                    for b in range(2):
                        for mq, n in mqs:
                            uev = uv[:, mq : mq + n, b, :, 0, :]
                            uod = uv[:, mq : mq + n, b, :, 1, :]
                            o0 = o[b][0][
                                :, mq * 512 : (mq + n) * 512
                            ].rearrange("m (mq hb w2) -> m mq hb w2", mq=n, hb=8)
                            o1 = o[b][1][
                                :, mq * 512 : (mq + n) * 512
                            ].rearrange("m (mq hb w2) -> m mq hb w2", mq=n, hb=8)
                            nc.vector.tensor_add(o0, uev, uod)
                            nc.vector.tensor_sub(o1, uev, uod)
                        if s == 1:
                            for g in range(2):
                                # SWDGE ring is otherwise idle; SP joins in
                                # once its input issues are done.
                                eng = (
                                    nc.sync
                                    if (pr == _PAIRS - 1 and g == 1)
                                    else nc.gpsimd
                                )
                                eng.dma_start(
                                    out=yv[b, g, pr, hh], in_=o[b][g][:, :]
                                )

                for m in range(4):  # mini-quads: hb in [8m, 8m+8)
                    pq = ppool.tile([128, 2048], f32, tag="pq", name=f"pq_{k}_{m}")
                    if h_on_dve:
                        # Single pass: P = M @ x, H left to DVE.
                        for j in range(4):
                            nc.tensor.matmul(
                                pq[:, j * 512 : (j + 1) * 512],
                                hmp,
                                xv2[:, 8 * m + 2 * j : 8 * m + 2 * j + 2],
                                start=True,
                                stop=True,
                            )
                    else:
                        # H butterfly via PSUM accumulation: q=0 pass (+M)
                        # then q=1 pass (+M into u0 half, -M into u1 half).
                        # LDWEIGHTS-friendly order: all +M matmuls first.
                        for b in range(2):
                            for c in range(2):  # bank: hb in [8m+4c, 8m+4c+4)
                                nc.tensor.matmul(
                                    pq[
                                        :,
                                        b * 1024 + c * 512 : b * 1024 + c * 512 + 512,
                                    ],
                                    hmp,
                                    xv[:, 0, 4 * (2 * m + c) : 4 * (2 * m + c) + 4],
                                    start=True,
                                    stop=False,
                                )
                        for b in range(2):
                            for c in range(2):
                                nc.tensor.matmul(
                                    pq[
                                        :,
                                        b * 1024 + c * 512 : b * 1024 + c * 512 + 512,
                                    ],
                                    (hmp, hmn)[b],
                                    xv[:, 1, 4 * (2 * m + c) : 4 * (2 * m + c) + 4],
                                    start=False,
                                    stop=True,
                                )
                    # PSUM f32 -> SBUF bf16 (ScalarE -- the 1x pacer).
                    nc.scalar.copy(ub[:, m * 2048 : (m + 1) * 2048], pq[:, :])
                    if m == 1:
                        w_stage(0)
                w_stage(1)
    nc.compile()
    return nc


_NC_CACHE = None


def _get_nc():
    global _NC_CACHE
    if _NC_CACHE is None:
        _NC_CACHE = _build_bass()
    return _NC_CACHE


def _run(x, trace=False, **spmd_kwargs):
    from concourse.bass_utils import run_bass_kernel_spmd

    x = np.ascontiguousarray(x, dtype=np.float32)
    xf = x.reshape(_SLABS, _D, _H, _W).astype(_BF16)
    M = _haar_matrix()
    Mpn = np.ascontiguousarray(np.concatenate([M, -M], axis=1)).astype(_BF16)
    in_maps = [
        {
            "x": np.ascontiguousarray(
                xf[i * _SLABS_PER_CORE : (i + 1) * _SLABS_PER_CORE]
            ),
            "hm": Mpn,
        }
        for i in range(_NCORES)
    ]
    res = run_bass_kernel_spmd(
        _get_nc(), in_maps, core_ids=list(range(_NCORES)), trace=trace, **spmd_kwargs
    )
    outs = [r["y"] for r in res.results]  # each (8, 4, 32, 64, 64) bf16
    full = np.concatenate(outs, axis=1).astype(np.float32)  # (8, 32, 32, 64, 64)
    full = full.reshape(8, _B, _C, _D // 2, _H // 2, _W // 2)
    return full, res


def kernel(**inputs):
    full, _ = _run(inputs["x"])
    return tuple(full[i] for i in range(8))


# revision 27
# speedup vs baseline: 1.0329x; 1.0329x over previous
"""3D Haar DWT (2x2x2 blocks, 8 subbands) on 8 Trainium2 NeuronCores.

Input  x: (2, 16, 64, 128, 128) f32.
Output: tuple of 8 subbands, each (2, 16, 32, 64, 64) f32, subband order
LLL,LLH,LHL,LHH,HLL,HLH,HHL,HHH (filters applied to (D,H,W) resp.).

Strategy (pure data parallel, zero cross-core communication), v2 = bf16:
  - The 2e-2 rel-err budget admits bf16 I/O (measured ~2e-3), halving HBM
    traffic per core to 8 MiB in + 8 MiB out -> ~47us DMA roofline/core.
  - Flatten (B,C) -> 32 slabs of (64,128,128); core i takes 4 = 2 PAIRS.
    A pair of slabs is processed together with SBUF partitions = (d, tp)
    (tp = slab parity).  Compared to (d, h-half) partitions this keeps
    per-partition DRAM runs at 16KB on input and 8KB on output (the h
    dimension stays whole in the free dim), so every DMA descriptor is
    large.
  - TensorEngine does the D-axis AND H-axis butterflies: the constant
    128x128 matrix M (and its negation) does the D butterfly on the
    partition axis, and PSUM accumulation over the two H-parity column
    sets does the H butterfly:
      u0 = M @ x[q=0] + M @ x[q=0],  u1 = M @ x[q=0] + (-M) @ x[q=1].
    The moving-operand access patterns feed columns in (hb, r, w2) order,
    so PSUM lands with the W-axis parity r DEINTERLEAVED.
  - ScalarE drains PSUM f32 -> SBUF bf16 (cast on copy) -- the only engine
    that can, at 1x (PSUM has one read port); it is the pipeline pacer.
  - DVE only does the W butterfly (pair r blocks, stride-1, bf16 2x mode).
  - A garbage-operand matmul warmup burst holds the PE busy ~4us up front
    so the HAM clock gate is at 2.4 GHz when real matmuls arrive.
  - Input DMAs on SP (HWDGE), output DMAs on GPSIMD (SWDGE): each ring has
    one producer so issues never queue behind each other.
"""

import numpy as np
import ml_dtypes

_B, _C, _D, _H, _W = 2, 16, 64, 128, 128
_NCORES = 8
_SLABS = _B * _C  # 32
_SLABS_PER_CORE = _SLABS // _NCORES  # 4
_PAIRS = _SLABS_PER_CORE // 2  # 2

_BF16 = ml_dtypes.bfloat16


def _haar_filters_np():
    # Bit-identical construction to the reference filter bank.
    s = 1.0 / np.sqrt(2.0)
    L = np.array([s, s], dtype=np.float32)
    H = np.array([s, -s], dtype=np.float32)
    bands = [(a, b, c) for a in "LH" for b in "LH" for c in "LH"]
    filt = np.stack(
        [
            (L if a == "L" else H)[:, None, None]
            * (L if b == "L" else H)[None, :, None]
            * (L if c == "L" else H)[None, None, :]
            for (a, b, c) in bands
        ],
        axis=0,
    )  # (8, 2, 2, 2) float32
    return filt


def _haar_matrix():
    """(128,128) for the D-axis butterfly on the partition axis.

    Input partition  = d*2 + tp          (tp = slab parity, d = depth 0..63)
    Output partition = tp*64 + dp*2 + a  (a = D band, dp = 0..31)
    (tp-major over dp so the output DMA's DRAM-side (tp, dp) dims merge into
    one 64-long OUTER dim -- 3-dim AP and a full 16-engine SDMA spray; an
    outer dim of 2 would put the whole transfer on 2 engines.)
    Entry = f_a[p] * s * s  (d = 2dp+p): the full 1/(2*sqrt2) magnitude is
    folded here so the H/W butterflies on DVE are pure +/- adds."""
    filt = _haar_filters_np()
    M = np.zeros((128, 128), dtype=np.float32)
    for tp in range(2):
        for a in range(2):
            for dp in range(32):
                for p in range(2):
                    M[(2 * dp + p) * 2 + tp, tp * 64 + dp * 2 + a] = filt[
                        a * 4, p, 0, 0
                    ]
    return M


def _build_bass():
    import concourse.mybir as mybir
    import concourse.tile as tile
    from concourse import bacc

    f32 = mybir.dt.float32
    bf16 = mybir.dt.bfloat16
    nc = bacc.Bacc("TRN2", target_bir_lowering=False, debug=False)

    x = nc.dram_tensor("x", [_SLABS_PER_CORE, _D, _H, _W], bf16, kind="ExternalInput")
    hm = nc.dram_tensor("hm", [128, 256], bf16, kind="ExternalInput")  # [M | -M]
    y = nc.dram_tensor(
        "y", [8, _SLABS_PER_CORE, _D // 2, _H // 2, _W // 2], bf16,
        kind="ExternalOutput",
    )

    # x[t=2pr+tp, d, h, w] with h = hh*64 + hb*2 + q, w = w2*2 + r.
    # Half-pair tile (pr, hh): partitions (d, tp), free (hb, q, w) -- each
    # partition's free dim walks a CONTIGUOUS 16KB HBM region; split into two
    # 1MB DMAs (hb halves, 8KB/partition each) so matmuls start early.
    xr = x[:, :, :, :].rearrange(
        "(pr tp) d (hh hb q) w -> pr d tp hh hb q w", tp=2, hh=2, hb=32, q=2
    )
    # y[s=(a,b,g), t=2pr+tp, dp, hp=(hh,hb), wp]; partition order (tp, dp, a);
    # one DMA per (b, g, pr, hh): DRAM dims ((tp dp):64, a:2, (hb wp):2048) --
    # 4KB contiguous per partition, 64-long outer dim for the engine spray.
    yv = y[:, :, :, :, :].rearrange(
        "(a b g) (pr tp) dp (hh hb) wp -> b g pr hh tp dp a hb wp",
        a=2, b=2, tp=2, hh=2,
    )

    with tile.TileContext(nc) as tc:
        with (
            tc.tile_pool(name="const", bufs=1) as cpool,
            tc.tile_pool(name="xin", bufs=3) as xpool,
            tc.tile_pool(name="uband", bufs=2) as upool,
            tc.tile_pool(name="outs", bufs=3) as opool,
            tc.tile_pool(name="psum", bufs=2, space="PSUM") as ppool,
        ):
            hmt = cpool.tile([128, 256], bf16, tag="hm")
            nc.sync.dma_start(out=hmt[:, :], in_=hm[:, :])
            hmp, hmn = hmt[:, 0:128], hmt[:, 128:256]

            # PE warmup: ~3us of garbage matmuls flips the HAM clock gate
            # to 8/8 (2.4 GHz) before the first real matmul; operands are a
            # memset tile so the burst starts at t~0, gated by nothing.
            junk = cpool.tile([128, 640], bf16, tag="junk")
            nc.vector.memset(junk[:, :], 0.0)
            wp = ppool.tile([128, 2048], f32, tag="pq", name="warm")
            for i in range(10):
                nc.tensor.matmul(
                    wp[:, 0:256], junk[:, 0:128], junk[:, 128:384],
                    start=True, stop=True,
                )

            def load_half_pair(pr, hh, nsplit=2):
                xh = xpool.tile([128, 8192], bf16, tag="xt", name=f"xt_{pr}_{hh}")
                w = 32 // nsplit
                for c in range(nsplit):
                    nc.sync.dma_start(
                        out=xh[:, c * w * 256 : (c + 1) * w * 256],
                        in_=xr[pr, :, :, hh, c * w : (c + 1) * w],
                    )
                return xh

            HP = [(pr, hh) for pr in range(_PAIRS) for hh in range(2)]
            # First tile in quarters so the first matmul chain starts ~1MB
            # earlier; the rest in halves (bigger descriptors).
            tiles = {HP[0]: load_half_pair(*HP[0], nsplit=4),
                     HP[1]: load_half_pair(*HP[1])}

            for k, (pr, hh) in enumerate(HP):
                if k + 2 < len(HP):
                    tiles[HP[k + 2]] = load_half_pair(*HP[k + 2])
                xt = tiles.pop((pr, hh))

                # Engine balance: half-pair 0 computes the H butterfly on DVE
                # (which would otherwise idle at the start); the rest fold H
                # into the PE via PSUM accumulation (2x matmul passes).
                h_on_dve = k == 0

                # Moving-operand APs, H parity q split out, remaining column
                # order (hb, r, w2): PSUM lands W-deinterleaved.
                xv = xt[:, :].rearrange(
                    "m (hb q w2 r) -> m q hb r w2", hb=32, q=2, r=2
                )
                # Single-pass variant: columns in (hb, q, r, w2) order.
                xv2 = xt[:, :].rearrange(
                    "m (hb q w2 r) -> m hb q r w2", hb=32, q=2, r=2
                )
                # u tile: post D+H data, free (mq: mini-quad 4, b: H band 2,
                # hb: 8, r: 2, w2: 64) -- one ScalarE drain per mini-quad.
                ub = upool.tile([128, 8192], bf16, tag="ub", name=f"ub_{pr}_{hh}")
                o = [
                    [
                        opool.tile(
                            [128, 2048], bf16, tag=f"o{b}{g}",
                            name=f"o{b}{g}_{pr}_{hh}",
                        )
                        for g in range(2)
                    ]
                    for b in range(2)
                ]

                if h_on_dve:
                    # ub holds P (pre-H) in (mq, hb, q, r, w2) layout; DVE's
                    # H butterfly (pair q blocks) writes ud in the standard
                    # (mq, b, hb, r, w2) layout the W stage expects.
                    ud = upool.tile(
                        [128, 8192], bf16, tag="ud", bufs=1, name=f"ud_{pr}_{hh}"
                    )
                    uv = ud[:, :].rearrange(
                        "m (mq b hb r w2) -> m mq b hb r w2", mq=4, b=2, hb=8, r=2
                    )
                    pv = ub[:, :].rearrange(
                        "m (mq hb q rw) -> m mq hb q rw", mq=4, hb=8, q=2
                    )
                    udh = ud[:, :].rearrange(
                        "m (mq b hb rw) -> m mq b hb rw", mq=4, b=2, hb=8
                    )
                else:
                    uv = ub[:, :].rearrange(
                        "m (mq b hb r w2) -> m mq b hb r w2", mq=4, b=2, hb=8, r=2
                    )

                def h_stage(s):
                    # H butterfly on DVE for semi s (h_on_dve half-pairs):
                    # pair q=0/q=1 blocks of (r, w2)=128, stride-1 bf16 (2x).
                    ev = pv[:, 2 * s : 2 * s + 2, :, 0, :]
                    od = pv[:, 2 * s : 2 * s + 2, :, 1, :]
                    u0 = udh[:, 2 * s : 2 * s + 2, 0, :, :]
                    u1 = udh[:, 2 * s : 2 * s + 2, 1, :, :]
                    nc.vector.tensor_add(u0, ev, od)
                    nc.vector.tensor_sub(u1, ev, od)

                last_hp = k == len(HP) - 1

                def w_stage(s):
                    # W butterfly on DVE for semi s (mini-quads 2s, 2s+1):
                    # pair r=0/r=1 blocks of w2=64, all stride-1 bf16 (2x).
                    # On the last half-pair run per-mini-quad so the final op
                    # chain after the last PSUM drain is as short as possible.
                    if h_on_dve:
                        h_stage(s)
                    mqs = [(2 * s, 2)]
                    for b in range(2):
                        for mq, n in mqs:
                            uev = uv[:, mq : mq + n, b, :, 0, :]
                            uod = uv[:, mq : mq + n, b, :, 1, :]
                            o0 = o[b][0][
                                :, mq * 512 : (mq + n) * 512
                            ].rearrange("m (mq hb w2) -> m mq hb w2", mq=n, hb=8)
                            o1 = o[b][1][
                                :, mq * 512 : (mq + n) * 512
                            ].rearrange("m (mq hb w2) -> m mq hb w2", mq=n, hb=8)
                            nc.vector.tensor_add(o0, uev, uod)
                            nc.vector.tensor_sub(o1, uev, uod)
                        if s == 1:
                            for g in range(2):
                                # SWDGE ring is otherwise idle; SP joins in
                                # once its input issues are done.
                                eng = (
                                    nc.sync
                                    if (pr == _PAIRS - 1 and g == 1)
                                    else nc.gpsimd
                                )
                                eng.dma_start(
                                    out=yv[b, g, pr, hh], in_=o[b][g][:, :]
                                )

                for m in range(4):  # mini-quads: hb in [8m, 8m+8)
                    pq = ppool.tile([128, 2048], f32, tag="pq", name=f"pq_{k}_{m}")
                    if h_on_dve:
                        # Single pass: P = M @ x, H left to DVE.
                        for j in range(4):
                            nc.tensor.matmul(
                                pq[:, j * 512 : (j + 1) * 512],
                                hmp,
                                xv2[:, 8 * m + 2 * j : 8 * m + 2 * j + 2],
                                start=True,
                                stop=True,
                            )
                    else:
                        # H butterfly via PSUM accumulation: q=0 pass (+M)
                        # then q=1 pass (+M into u0 half, -M into u1 half).
                        # LDWEIGHTS-friendly order: all +M matmuls first.
                        for b in range(2):
                            for c in range(2):  # bank: hb in [8m+4c, 8m+4c+4)
                                nc.tensor.matmul(
                                    pq[
                                        :,
                                        b * 1024 + c * 512 : b * 1024 + c * 512 + 512,
                                    ],
                                    hmp,
                                    xv[:, 0, 4 * (2 * m + c) : 4 * (2 * m + c) + 4],
                                    start=True,
                                    stop=False,
                                )
                        for b in range(2):
                            for c in range(2):
                                nc.tensor.matmul(
                                    pq[
                                        :,
                                        b * 1024 + c * 512 : b * 1024 + c * 512 + 512,
                                    ],
                                    (hmp, hmn)[b],
                                    xv[:, 1, 4 * (2 * m + c) : 4 * (2 * m + c) + 4],
                                    start=False,
                                    stop=True,
                                )
                    # PSUM f32 -> SBUF bf16 (ScalarE -- the 1x pacer).
                    nc.scalar.copy(ub[:, m * 2048 : (m + 1) * 2048], pq[:, :])
                    if m == 1:
                        w_stage(0)
                w_stage(1)
    nc.compile()
    return nc


_NC_CACHE = None


def _get_nc():
    global _NC_CACHE
    if _NC_CACHE is None:
        _NC_CACHE = _build_bass()
    return _NC_CACHE


def _run(x, trace=False, **spmd_kwargs):
    from concourse.bass_utils import run_bass_kernel_spmd

    x = np.ascontiguousarray(x, dtype=np.float32)
    xf = x.reshape(_SLABS, _D, _H, _W).astype(_BF16)
    M = _haar_matrix()
    Mpn = np.ascontiguousarray(np.concatenate([M, -M], axis=1)).astype(_BF16)
    in_maps = [
        {
            "x": np.ascontiguousarray(
                xf[i * _SLABS_PER_CORE : (i + 1) * _SLABS_PER_CORE]
            ),
            "hm": Mpn,
        }
        for i in range(_NCORES)
    ]
    res = run_bass_kernel_spmd(
        _get_nc(), in_maps, core_ids=list(range(_NCORES)), trace=trace, **spmd_kwargs
    )
    outs = [r["y"] for r in res.results]  # each (8, 4, 32, 64, 64) bf16
    full = np.concatenate(outs, axis=1).astype(np.float32)  # (8, 32, 32, 64, 64)
    full = full.reshape(8, _B, _C, _D // 2, _H // 2, _W // 2)
    return full, res


def kernel(**inputs):
    full, _ = _run(inputs["x"])
    return tuple(full[i] for i in range(8))
